# revision 1
# baseline (speedup 1.0000x reference)
"""GAT kernel for 8 Trainium2 NeuronCores — v5.

1D dst-partitioning (one SPMD program, rotated per-core node tables, edges
bucketed by dst chunk of 128 with counts padded to the cross-core max):
  * Phase 1: htab[r] = [h(256) | a_src(4) | a_dst(4)] f16 in 768B rows,
    split into two 25000-row halves (int16 gather indices).
  * Phase 2 per chunk: dma_gather edge source rows (f16, 768B), one-hot
    dst masks scatter-accumulate weighted messages + softmax denominators
    into PSUM via matmul; softmax division, bias, relu, L2-norm on the tail.

Performance structure:
  * dma_gather descriptor-gen on the 8 Q7s is the critical resource
    (~8ns/row on one SWDGE queue).  Calls rotate across 4 SWDGE queues,
    whose generation overlaps (~3x), 8 tiles (1024 rows) per call.
  * Pad slots gather table row 0 (finite data; their dstl=-1 zero-masks
    them out of the scatter).  Gather buffers are memset once for safety.
  * maskT comes from is_eq against a partition-broadcast DMA of the dst-row
    ids (all-f16, packed, 2x DVE) instead of a PE broadcast matmul (the v1
    drp4), and per-edge a_dst rides a per-tile [128x128]x[128x4] matmul.
  * Self-loop edges are not materialized: every chunk adds its own nodes'
    self contribution in the tail from table rows directly.
"""

import sys

sys.path.insert(0, "/opt/trn_rl_repo")

import numpy as np

HEADS = 4
OUT_CH = 64
NEG_SLOPE = 0.2
P = 128
AUG = 256 + 2 * HEADS  # 264
ROW16 = 384  # f16 elems per table row (768 B, multiple of 256)


# --------------------------------------------------------------------------
# host-side preprocessing (sharding + layout only, plus parameter fusion)
# --------------------------------------------------------------------------
def _preprocess(x, edge_index, W, att_src, att_dst, bias, n_cores):
    x = np.asarray(x, np.float32)
    N, IN = x.shape
    assert N % n_cores == 0 and N % 2 == 0
    npc = N // n_cores
    half = N // 2
    assert half <= 32767
    chunks = (npc + P - 1) // P

    # self loops are handled analytically in the chunk tail, not as edges
    src = np.asarray(edge_index[0], np.int64)
    dst = np.asarray(edge_index[1], np.int64)

    core = dst // npc
    rem = dst - core * npc
    chunk = rem // P
    dstl = (rem - chunk * P).astype(np.float32)

    per_core = []
    for k in range(n_cores):
        sel = np.nonzero(core == k)[0]
        loc = (src[sel] - k * npc) % N  # rotated table row of the source
        hlf = (loc >= half).astype(np.int64)
        idx16 = (loc - hlf * half).astype(np.int16)
        key = chunk[sel] * 2 + hlf
        order = np.argsort(key, kind="stable")
        counts = np.bincount(key, minlength=chunks * 2).reshape(chunks, 2)
        starts = np.zeros(chunks * 2 + 1, np.int64)
        np.cumsum(counts.reshape(-1), out=starts[1:])
        per_core.append((idx16[order], dstl[sel][order], counts, starts))

    all_counts = np.stack([pc[2] for pc in per_core])  # [cores, chunks, 2]
    Tch = np.maximum(1, -(-all_counts.max(axis=0) // P))  # [chunks, 2]
    slots_per_chunk = P * (Tch[:, 0] + Tch[:, 1])
    total_slots = int(slots_per_chunk.sum())
    TT = int(total_slots // P)
    S16 = total_slots // 16

    chunk_off = np.zeros(chunks + 1, np.int64)
    np.cumsum(slots_per_chunk, out=chunk_off[1:])

    import os as _os

    _negpad = _os.environ.get("GAT_NEGPAD", "0") == "1"
    idx_pad = np.full(
        (n_cores, total_slots), -1 if _negpad else 0, np.int16
    )  # -1: Q7-trimmed
    dstl_pad = np.full((n_cores, total_slots), -1.0, np.float32)
    for k in range(n_cores):
        idx_s, dstl_s, counts, starts = per_core[k]
        for c in range(chunks):
            off = int(chunk_off[c])
            for h in range(2):
                g = c * 2 + h
                s0, s1 = int(starts[g]), int(starts[g + 1])
                n = s1 - s0
                idx_pad[k, off : off + n] = idx_s[s0:s1]
                dstl_pad[k, off : off + n] = dstl_s[s0:s1]
                off += int(P * Tch[c, h])

    # wrap gather indices: within each (chunk, half) group, idx j -> [j%16, j//16]
    idxs_w = np.zeros((n_cores, 16, S16), np.int16)
    for c in range(chunks):
        off = int(chunk_off[c])
        for h in range(2):
            G = int(P * Tch[c, h])
            blk = idx_pad[:, off : off + G].reshape(n_cores, G // 16, 16)
            idxs_w[:, :, off // 16 : (off + G) // 16] = blk.transpose(0, 2, 1)
            off += G
    idxs_rep = np.ascontiguousarray(np.tile(idxs_w, (1, 8, 1)))  # 8 Q7 cores

    dcol = np.ascontiguousarray(
        dstl_pad.reshape(n_cores, TT, P).transpose(0, 2, 1)
    ).astype(np.float16)  # [cores, 128, TT]
    drow = np.ascontiguousarray(dstl_pad).astype(np.float16)  # [cores, TT*128]

    # parameter-only fusion: a_src = h @ att_src == x @ w_src
    W4 = np.asarray(W, np.float32).reshape(IN, HEADS, OUT_CH)
    w_src = np.einsum("ihc,hc->ih", W4, np.asarray(att_src, np.float32))
    w_dst = np.einsum("ihc,hc->ih", W4, np.asarray(att_dst, np.float32))
    Waug = np.ascontiguousarray(
        np.concatenate([np.asarray(W, np.float32), w_src, w_dst], axis=1)
    ).astype(np.float16)  # [IN, AUG]

    xT = np.ascontiguousarray(x.T).astype(np.float16)  # [IN, N]

    meta = dict(
        N=N,
        IN=IN,
        npc=npc,
        half=half,
        chunks=chunks,
        Tch=Tch,
        chunk_off=chunk_off,
        TT=TT,
        S16=int(S16),
    )
    in_maps = []
    for k in range(n_cores):
        rot = np.roll(np.arange(N), -k * npc)  # table row r -> node rot[r]
        in_maps.append(
            {
                "xT": np.ascontiguousarray(xT[:, rot]),
                "Waug": Waug,
                "idxs": idxs_rep[k],
                "dcol": dcol[k],
                "drow": drow[k],
                "bias": np.asarray(bias, np.float32),
            }
        )
    return meta, in_maps


# --------------------------------------------------------------------------
# device program (identical on every core)
# --------------------------------------------------------------------------
def _build_program(meta, n_cores, debug=False):
    import os

    import concourse.bacc as bacc
    import concourse.mybir as mybir
    import concourse.tile as tile

    GG = int(os.environ.get("GAT_G", "8"))  # edge-tiles per dma_gather call
    NQ = int(os.environ.get("GAT_NQ", "4"))  # SWDGE queues (gens overlap)
    GB = int(os.environ.get("GAT_GBUFS", "8"))  # gather buffers in flight

    f32 = mybir.dt.float32
    f16 = mybir.dt.float16
    i16 = mybir.dt.int16

    N, IN = meta["N"], meta["IN"]
    npc, half, chunks = meta["npc"], meta["half"], meta["chunks"]
    Tch, chunk_off = meta["Tch"], meta["chunk_off"]
    TT, S16 = meta["TT"], meta["S16"]
    KB = IN // P  # contraction blocks (2)
    n_ntiles = (N + P - 1) // P

    nc = bacc.Bacc(
        "TRN2",
        target_bir_lowering=False,
        debug=debug,
        num_devices=n_cores,
        num_swdge_queues=NQ,
        dynamic_dma_scratch_size=int(os.environ.get("GAT_SCRATCH", "16384")),
    )

    def mm(out, lhsT, rhs, **kw):
        nc.tensor.matmul(out, lhsT, rhs, **kw)

    xT_d = nc.dram_tensor("xT", [IN, N], f16, kind="ExternalInput")
    Waug_d = nc.dram_tensor("Waug", [IN, AUG], f16, kind="ExternalInput")
    idxs_d = nc.dram_tensor("idxs", [P, S16], i16, kind="ExternalInput")
    dcol_d = nc.dram_tensor("dcol", [P, TT], f16, kind="ExternalInput")
    drow_d = nc.dram_tensor("drow", [TT * P], f16, kind="ExternalInput")
    bias_d = nc.dram_tensor("bias", [IN], f32, kind="ExternalInput")
    out_d = nc.dram_tensor("out", [npc, IN], f32, kind="ExternalOutput")
    htab_lo = nc.dram_tensor("htab_lo", [half, ROW16], f16)
    htab_hi = nc.dram_tensor("htab_hi", [half, ROW16], f16)

    with tile.TileContext(nc) as tc:
        with tc.tile_pool(name="const", bufs=1) as cpool:
            iota_row8 = cpool.tile([P, 8, P], f16)
            nc.gpsimd.iota(
                iota_row8[:],
                pattern=[[0, 8], [1, P]],
                base=0,
                channel_multiplier=0,
                allow_small_or_imprecise_dtypes=True,
            )
            iota_col8 = cpool.tile([P, 8, P], f16)
            nc.gpsimd.iota(
                iota_col8[:],
                pattern=[[0, 8], [0, P]],
                base=0,
                channel_multiplier=1,
                allow_small_or_imprecise_dtypes=True,
            )
            ones_row = cpool.tile([1, P], f32)
            nc.vector.memset(ones_row[:], 1.0)

            bias_row = cpool.tile([1, IN], f32)
            nc.sync.dma_start(out=bias_row[:], in_=bias_d[None, :])
            bias_full = cpool.tile([P, HEADS, OUT_CH], f32)
            with tc.tile_pool(name="cpsum", bufs=1, space="PSUM") as cpsum:
                bias_psum = cpsum.tile([P, HEADS, OUT_CH], f32)
                nc.tensor.matmul(
                    bias_psum[:], ones_row[:], bias_row[:], start=True, stop=True
                )
                nc.vector.tensor_copy(bias_full[:], bias_psum[:])

            Waug_sb = cpool.tile([P, KB, AUG], f16)
            for k in range(KB):
                nc.sync.dma_start(
                    out=Waug_sb[:, k, :], in_=Waug_d[k * P : (k + 1) * P, :]
                )

            # ------------------------------------------------------------
            # phase 1: htab[r] = [h | a_src | a_dst] (f16 rows)
            # ------------------------------------------------------------
            with (
                tc.tile_pool(name="xload", bufs=3) as xpool,
                tc.tile_pool(name="hout", bufs=4) as hpool,
                tc.tile_pool(name="hpsum", bufs=4, space="PSUM") as hpsum,
            ):
                NB1 = 8  # node tiles per x load
                for nt0 in range(0, n_ntiles, NB1):
                    nbt = min(NB1, n_ntiles - nt0)
                    n00 = nt0 * P
                    pall = min(NB1 * P, N - n00)
                    xt = xpool.tile([P, KB, NB1 * P], f16)
                    for k in range(KB):
                        nc.sync.dma_start(
                            out=xt[:, k, :pall],
                            in_=xT_d[k * P : (k + 1) * P, n00 : n00 + pall],
                        )
                    for j in range(nbt):
                        n0 = n00 + j * P
                        p = min(P, N - n0)
                        hp = hpsum.tile([P, AUG], f32)
                        for k in range(KB):
                            mm(
                                hp[:p, :],
                                xt[:, k, j * P : j * P + p],
                                Waug_sb[:, k, :],
                                start=(k == 0),
                                stop=(k == KB - 1),
                            )
                        hs = hpool.tile([P, AUG], f16)
                        nc.vector.tensor_copy(hs[:p, :], hp[:p, :])
                        if n0 + p <= half:
                            nc.scalar.dma_start(
                                out=htab_lo[n0 : n0 + p, 0:AUG], in_=hs[:p, :]
                            )
                        elif n0 >= half:
                            nc.scalar.dma_start(
                                out=htab_hi[n0 - half : n0 - half + p, 0:AUG],
                                in_=hs[:p, :],
                            )
                        else:
                            pl = half - n0
                            nc.scalar.dma_start(
                                out=htab_lo[n0:half, 0:AUG], in_=hs[:pl, :]
                            )
                            nc.scalar.dma_start(
                                out=htab_hi[0 : n0 + p - half, 0:AUG],
                                in_=hs[pl:p, :],
                            )

            # ------------------------------------------------------------
            # phase 2: per dst-chunk edge aggregation
            # ------------------------------------------------------------
            with (
                tc.tile_pool(name="gath", bufs=GB) as gpool,
                tc.tile_pool(name="meta2", bufs=2) as mpool,
                tc.tile_pool(name="work", bufs=4) as wpool,
                tc.tile_pool(name="masks", bufs=4) as kpool,
                tc.tile_pool(name="rhs", bufs=4) as rpool,
                tc.tile_pool(name="tail", bufs=2) as fpool,
                tc.tile_pool(name="opsum", bufs=2, space="PSUM") as opsum,
                tc.tile_pool(name="apsum", bufs=2, space="PSUM") as apsum,
            ):
                # pre-finite the gather buffers: trailing pad slots (idx=-1)
                # are skipped by the Q7 kernel and must hold finite stale data
                for _ in range(GB):
                    z = gpool.tile([P, GG, ROW16], f16, tag="ggb")
                    nc.vector.memset(z[:], 0.0)

                qrot = [0]
                for c in range(chunks):
                    T0, T1 = int(Tch[c, 0]), int(Tch[c, 1])
                    Tc = T0 + T1
                    toff = int(chunk_off[c]) // P
                    s16 = int(chunk_off[c]) // 16
                    pc = min(P, npc - c * P)

                    dcol_sb = mpool.tile([P, Tc], f16, tag="dcol")
                    nc.sync.dma_start(
                        out=dcol_sb[:], in_=dcol_d[:, toff : toff + Tc]
                    )
                    drow_bc = mpool.tile([P, Tc * P], f16, tag="drowbc")
                    nc.scalar.dma_start(
                        out=drow_bc[:],
                        in_=drow_d[toff * P : (toff + Tc) * P][
                            None, :
                        ].to_broadcast([P, Tc * P]),
                    )
                    idx_sb = mpool.tile([P, (Tc * P) // 16], i16, tag="idx")
                    nc.sync.dma_start(
                        out=idx_sb[:], in_=idxs_d[:, s16 : s16 + (Tc * P) // 16]
                    )
                    # a_dst of this chunk's own dst nodes (table rows c*128..)
                    adst_sb = mpool.tile([P, HEADS], f16, tag="adst")
                    nc.vector.memset(adst_sb[:], 0.0)
                    nc.sync.dma_start(
                        out=adst_sb[:pc, :],
                        in_=htab_lo[
                            c * P : c * P + pc, IN + HEADS : IN + 2 * HEADS
                        ],
                    )
                    # own rows for the self-loop contribution in the tail
                    hd = fpool.tile([P, AUG], f16, tag="hd")
                    nc.vector.memset(hd[:], 0.0)
                    nc.scalar.dma_start(
                        out=hd[:pc, :], in_=htab_lo[c * P : c * P + pc, 0:AUG]
                    )

                    out_ps = opsum.tile([P, HEADS, 65], f32)
                    for hh, (Th, t0, tab) in enumerate(
                        (
                            (T0, 0, htab_lo[:, :]),
                            (T1, T0, htab_hi[:, :]),
                        )
                    ):
                        ib = (T0 * P) // 16 if hh else 0
                        for g0 in range(0, Th, GG):
                            nb = min(GG, Th - g0)
                            t = t0 + g0
                            ggb = gpool.tile([P, GG, ROW16], f16, tag="ggb")
                            nc.gpsimd.dma_gather(
                                ggb[:, :nb, :],
                                tab,
                                idx_sb[:, ib + g0 * 8 : ib + (g0 + nb) * 8],
                                nb * P,
                                nb * P,
                                ROW16,
                                queue_num=qrot[0] % NQ,
                            )
                            qrot[0] += 1
                            gv = ggb[:, :nb, :]
                            brhs = rpool.tile([P, 8, HEADS, 65], f16, tag="grhs")
                            mask8 = kpool.tile([P, 8, P], f16, tag="mask")
                            nc.vector.tensor_tensor(
                                out=mask8[:, :nb, :],
                                in0=dcol_sb[:, t : t + nb][
                                    :, :, None
                                ].to_broadcast([P, nb, P]),
                                in1=iota_row8[:, :nb, :],
                                op=mybir.AluOpType.is_equal,
                            )
                            maskT8 = kpool.tile([P, 8, P], f16, tag="maskT")
                            nc.vector.tensor_tensor(
                                out=maskT8[:, :nb, :],
                                in0=iota_col8[:, :nb, :],
                                in1=drow_bc[:, t * P : (t + nb) * P].rearrange(
                                    "p (g q) -> p g q", q=P
                                ),
                                op=mybir.AluOpType.is_equal,
                            )
                            aep8 = apsum.tile([P, 8, HEADS], f32)
                            for i in range(nb):
                                mm(
                                    aep8[:, i, :],
                                    maskT8[:, i, :],
                                    adst_sb[:],
                                    start=True,
                                    stop=True,
                                )
                            e08 = wpool.tile([P, 8, HEADS], f32, tag="e0")
                            nc.vector.tensor_add(
                                e08[:, :nb, :],
                                gv[:, :, IN : IN + HEADS],
                                aep8[:, :nb, :],
                            )
                            epos8 = wpool.tile([P, 8, HEADS], f32, tag="ep")
                            nc.scalar.activation(
                                epos8[:, :nb, :],
                                e08[:, :nb, :],
                                mybir.ActivationFunctionType.Relu,
                                scale=1.0 - NEG_SLOPE,
                            )
                            el8 = wpool.tile([P, 8, HEADS], f32, tag="el")
                            nc.scalar.activation(
                                el8[:, :nb, :],
                                e08[:, :nb, :],
                                mybir.ActivationFunctionType.Copy,
                                scale=NEG_SLOPE,
                            )
                            nc.vector.tensor_add(
                                el8[:, :nb, :],
                                el8[:, :nb, :],
                                epos8[:, :nb, :],
                            )
                            nc.scalar.activation(
                                brhs[:, :nb, :, 64],
                                el8[:, :nb, :],
                                mybir.ActivationFunctionType.Exp,
                            )
                            # packed f16 exp weights: every multiply operand
                            # is then 2-byte stride-1, unlocking DVE 2X
                            wexpx = rpool.tile(
                                [P, GG, HEADS, OUT_CH], f16, tag="wexpx"
                            )
                            nc.scalar.activation(
                                wexpx[:, :nb, :, :],
                                el8[:, :nb, :, None].to_broadcast(
                                    [P, nb, HEADS, OUT_CH]
                                ),
                                mybir.ActivationFunctionType.Exp,
                            )
                            nc.vector.tensor_tensor(
                                out=brhs[:, :nb, :, 0:64],
                                in0=gv[:, :, 0:IN].rearrange(
                                    "p g (h c) -> p g h c", h=HEADS
                                ),
                                in1=wexpx[:, :nb, :, :],
                                op=mybir.AluOpType.mult,
                            )
                            for i in range(nb):
                                mm(
                                    out_ps[:],
                                    mask8[:, i, :],
                                    brhs[:, i],
                                    start=(t + i == 0),
                                    stop=(t + i == Tc - 1),
                                )
                    # ---- chunk tail: self loop, softmax div, bias, relu, L2
                    e0s = fpool.tile([P, HEADS], f32, tag="e0s")
                    nc.vector.tensor_add(
                        e0s[:], hd[:, IN : IN + HEADS],
                        hd[:, IN + HEADS : IN + 2 * HEADS],
                    )
                    eps = fpool.tile([P, HEADS], f32, tag="eps")
                    nc.scalar.activation(
                        eps[:], e0s[:],
                        mybir.ActivationFunctionType.Relu,
                        scale=1.0 - NEG_SLOPE,
                    )
                    els = fpool.tile([P, HEADS], f32, tag="els")
                    nc.scalar.activation(
                        els[:], e0s[:],
                        mybir.ActivationFunctionType.Copy,
                        scale=NEG_SLOPE,
                    )
                    nc.vector.tensor_add(els[:], els[:], eps[:])
                    es = fpool.tile([P, HEADS], f32, tag="es")
                    nc.scalar.activation(
                        es[:], els[:], mybir.ActivationFunctionType.Exp
                    )
                    sm = fpool.tile([P, HEADS, OUT_CH], f32, tag="sm")
                    nc.vector.tensor_tensor(
                        out=sm[:],
                        in0=hd[:, 0:IN].rearrange("p (h c) -> p h c", h=HEADS),
                        in1=es[:, :, None].to_broadcast([P, HEADS, OUT_CH]),
                        op=mybir.AluOpType.mult,
                    )
                    dn = fpool.tile([P, HEADS], f32, tag="dn")
                    nc.vector.tensor_add(dn[:], out_ps[:, :, 64], es[:])
                    nc.vector.tensor_scalar_max(dn[:], dn[:], 1e-30)
                    rdn = fpool.tile([P, HEADS], f32, tag="rdn")
                    nc.vector.reciprocal(rdn[:], dn[:])
                    o1 = fpool.tile([P, HEADS, OUT_CH], f32, tag="o1")
                    nc.vector.tensor_add(o1[:], out_ps[:, :, 0:64], sm[:])
                    nc.vector.tensor_tensor(
                        out=o1[:],
                        in0=o1[:],
                        in1=rdn[:, :, None].to_broadcast([P, HEADS, OUT_CH]),
                        op=mybir.AluOpType.mult,
                    )
                    nc.vector.tensor_add(o1[:], o1[:], bias_full[:])
                    o2 = fpool.tile([P, HEADS, OUT_CH], f32, tag="o2")
                    nc.scalar.activation(
                        o2[:], o1[:], mybir.ActivationFunctionType.Relu
                    )
                    sq = fpool.tile([P, HEADS, OUT_CH], f32, tag="sq")
                    nc.vector.tensor_mul(sq[:], o2[:], o2[:])
                    s = fpool.tile([P, 1], f32, tag="s")
                    nc.vector.tensor_reduce(
                        s[:],
                        sq[:],
                        axis=mybir.AxisListType.XY,
                        op=mybir.AluOpType.add,
                    )
                    r = fpool.tile([P, 1], f32, tag="r")
                    nc.scalar.sqrt(r[:], s[:])
                    nc.vector.tensor_scalar_max(r[:], r[:], 1e-12)
                    rr = fpool.tile([P, 1], f32, tag="rr")
                    nc.vector.reciprocal(rr[:], r[:])
                    o3 = fpool.tile([P, HEADS, OUT_CH], f32, tag="o3")
                    nc.vector.tensor_scalar_mul(o3[:], o2[:], rr[:])
                    nc.sync.dma_start(
                        out=out_d[c * P : c * P + pc, :], in_=o3[:pc]
                    )

    nc.compile()
    return nc


# --------------------------------------------------------------------------
# entry point: full inputs in, full output out
# --------------------------------------------------------------------------
def kernel(x, edge_index, W, att_src, att_dst, bias):
    from concourse.bass_utils import run_bass_kernel_spmd

    n_cores = 8
    meta, in_maps = _preprocess(
        x, edge_index, W, att_src, att_dst, bias, n_cores
    )
    nc = _build_program(meta, n_cores)
    res = run_bass_kernel_spmd(nc, in_maps, list(range(n_cores)))
    out = np.concatenate(
        [res.results[k]["out"] for k in range(n_cores)], axis=0
    )
    return out.astype(np.float32)

